# revision 1
# baseline (speedup 1.0000x reference)
"""MemoryReader kernel for Trainium2, data-parallel over batch across 8 cores.

Per batch element b (one NeuronCore each):
    mkf = mk[b] as [CK=64, M=4096], qkf = qk[b] as [CK, N=4096]
    aff[m, n] = (2 * mkf.T @ qkf - |mkf[:,m]|^2) / sqrt(CK)
    P = softmax over m
    mem[c, n]  = sum_m mv[b][c, m] * P[m, n]
    out[b] = concat([mem, qv[b]], channel axis)

Device kernel layout (per core):
    - QK^T matmuls produce aff tiles in [m-partition, n-free] layout,
      32 m-chunks of [128, 512] per n-super-tile of 512 columns.
    - ScalarE computes E = exp(0.25*ab - a_sq/8) straight out of PSUM
      (per-partition bias = -a_sq/8; logits are bounded so the max
      subtraction of a standard softmax is unnecessary in fp32).
    - VectorE accumulates sum_m E chunk-by-chunk; a ones-vector matmul
      folds the partition axis; reciprocal + DMA partition-broadcast
      give 1/s replicated across partitions.
    - Readout matmuls contract over m in PSUM (4 c-chunks of 128), then
      VectorE scales by 1/s while evacuating PSUM.
    - mv^T / mk^T are prepared host-side (pure layout transforms), so no
      on-device transposes are needed. qv never touches the device.
"""

import os
import sys

import numpy as np

B, CK, CV, H, W = 8, 64, 512, 64, 64
M = H * W          # memory positions per batch element
N = H * W          # query positions
NT = 512           # n-super-tile width (columns per softmax pass)
NSUP = N // NT     # 8 n-super-tiles
MCH = M // 128     # 32 m-chunks
N_CORES = 8

# "fp32r" runs matmuls in relaxed-precision single-pass mode (4x faster
# than exact fp32 on the PE array); "fp32" is exact.
MATMUL_PREC = os.environ.get("KERNEL_MATMUL_PREC", "fp32r")

_CACHE = {}


def _build_program():
    sys.path.insert(0, "/opt/trn_rl_repo")
    from contextlib import ExitStack

    import concourse.tile as tile
    from concourse import bacc, mybir

    dt = mybir.dt
    f32 = dt.float32
    # Matmul operand dtype: float32r (relaxed single-pass fp32, 4x faster
    # on the PE array) or exact float32. Bit-layout is identical; walrus
    # requires producers of fp32r matmul operands to be typed fp32r.
    mdt = dt.float32r if MATMUL_PREC == "fp32r" else f32

    nc = bacc.Bacc("TRN2", target_bir_lowering=False, debug=False,
                   num_devices=N_CORES)

    mk_d = nc.dram_tensor("mk", [128, M], mdt, kind="ExternalInput").ap()
    mkt_d = nc.dram_tensor("mkt", [128, MCH * CK], f32,
                           kind="ExternalInput").ap()
    qk_d = nc.dram_tensor("qk", [128, N], mdt, kind="ExternalInput").ap()
    mvt_d = nc.dram_tensor("mvt", [MCH, 128, CV], mdt,
                           kind="ExternalInput").ap()
    mem_d = nc.dram_tensor("mem", [CV, N], f32, kind="ExternalOutput").ap()

    with tile.TileContext(nc) as tc, ExitStack() as ctx:
        sing = ctx.enter_context(tc.tile_pool(name="sing", bufs=1))
        e_pool = ctx.enter_context(tc.tile_pool(name="E", bufs=17))
        scratch = ctx.enter_context(tc.tile_pool(name="scratch", bufs=2))
        sacc_pool = ctx.enter_context(tc.tile_pool(name="sacc", bufs=2))
        row_pool = ctx.enter_context(tc.tile_pool(name="row", bufs=2))
        rb_pool = ctx.enter_context(tc.tile_pool(name="rb", bufs=2))
        out_pool = ctx.enter_context(tc.tile_pool(name="out", bufs=8))
        qk_ps_pool = ctx.enter_context(
            tc.tile_pool(name="qkps", bufs=2, space="PSUM"))
        ro_ps_pool = ctx.enter_context(
            tc.tile_pool(name="rops", bufs=1, space="PSUM"))


        # PE warmup: the PE activity monitor starts throttled at 1.2 GHz
        # and needs ~3.4us of sustained matmul activity to unthrottle.
        # Burn dummy matmuls while the input DMAs stream so the real
        # matmuls start at 2.4 GHz.
        warm_sb = sing.tile([128, NT], f32)
        nc.vector.memset(warm_sb[:], 1.0)
        warm_ps = qk_ps_pool.tile([128, NT], f32, tag="qk_ps", name="warm_ps")
        for w in range(56):
            nc.tensor.matmul(warm_ps[:, 0:128], lhsT=warm_sb[:, 0:128],
                             rhs=warm_sb[:, 0:128], start=True, stop=True)

        # Resident inputs. mk/qk are zero-padded from CK=64 to K=128
        # contraction rows: K=64 matmuls leave the PE activity monitor
        # throttled at 1.2 GHz (measured 427 ns/MM vs 222 ns at K=128),
        # so padded K=128 matmuls are 2x faster despite wasting rows.
        # All DMAs go through the sync engine (hardware DGE); ordered so
        # the tensors gating the first matmuls arrive first.
        mk_sb = sing.tile([128, M], mdt)
        qk_sb = sing.tile([128, N], mdt)
        mkt_sb = sing.tile([128, MCH, CK], f32)
        mvt_sb = sing.tile([128, MCH, CV], mdt)
        for g in range(4):
            gs = slice(g * 1024, (g + 1) * 1024)
            nc.sync.dma_start(out=mk_sb[:, gs], in_=mk_d[:, gs])
        nc.sync.dma_start(out=qk_sb[:, 0:NT], in_=qk_d[:, 0:NT])
        nc.sync.dma_start(out=mkt_sb[:], in_=mkt_d[:].rearrange(
            "p (j c) -> p j c", c=CK))
        for j in range(4):
            nc.sync.dma_start(out=mvt_sb[:, j, :], in_=mvt_d[j])
        nc.sync.dma_start(out=qk_sb[:, NT:N], in_=qk_d[:, NT:N])
        for j in range(4, MCH):
            nc.sync.dma_start(out=mvt_sb[:, j, :], in_=mvt_d[j])

        # Ones vectors typed fp32r so the softmax-sum and broadcast
        # matmuls take the single-pass PE path (213 ns vs 853 ns).
        ones_f32 = sing.tile([128, 1], f32)
        nc.vector.memset(ones_f32[:], 1.0)
        ones_sb = sing.tile([128, 1], mdt)
        nc.vector.tensor_copy(ones_sb[:], ones_f32[:].bitcast(mdt))
        ones_row_f32 = sing.tile([1, 128], f32)
        nc.vector.memset(ones_row_f32[:], 1.0)
        ones_row = sing.tile([1, 128], mdt)
        nc.vector.tensor_copy(ones_row[:], ones_row_f32[:].bitcast(mdt))

        # Per-partition softmax bias: asq[p, j] = -|mk[:, j*128+p]|^2 / 8.
        # (tensor_tensor_reduce crashes on HW via this toolchain; use
        # Square -> free-axis reduce -> scale, in 4 pieces to keep the
        # scratch small.)
        asq = sing.tile([128, MCH], f32)
        for piece in range(4):
            js = slice(piece * 8, (piece + 1) * 8)
            sqp = scratch.tile([128, 8, CK], f32, tag="sqp",
                               name=f"sqp{piece}")
            nc.scalar.activation(sqp[:], mkt_sb[:, js, :],
                                 mybir.ActivationFunctionType.Square)
            nc.vector.tensor_reduce(asq[:, js], sqp[:],
                                    axis=mybir.AxisListType.X,
                                    op=mybir.AluOpType.add)
        nc.scalar.mul(asq[:], asq[:], -0.125)
        # g[p, j] = exp(-|mk row|^2 / 8); folded into the value rows and
        # the denominator accumulation so the exp needs no bias and can
        # span two PSUM banks per instruction.
        g_col = sing.tile([128, MCH], f32)
        nc.scalar.activation(g_col[:], asq[:],
                             mybir.ActivationFunctionType.Exp)
        with nc.allow_low_precision(reason="fp32r is fp32 bits"):
            for j in range(MCH):
                nc.vector.tensor_scalar_mul(mvt_sb[:, j, :],
                                            mvt_sb[:, j, :],
                                            g_col[:, j:j + 1])

        def emit_tail(ti, tsacc, tosbs, tnsl):
            # Softmax denominator, reciprocal, partition-broadcast and
            # final scaling for super `ti`. Emitted a few chunks into the
            # NEXT super so the PE stream has QK matmuls to chew on while
            # the DVE-side reduction chain resolves.
            s_ps = qk_ps_pool.tile([1, NT], f32, tag="qk_ps",
                                   name=f"sps{ti}")
            nc.tensor.matmul(s_ps[:], lhsT=ones_sb[:], rhs=tsacc[:],
                             start=True, stop=True)
            s_row = row_pool.tile([1, NT], mdt, tag="srow",
                                  name=f"srow{ti}")
            with nc.allow_low_precision(reason="fp32r is fp32 bits"):
                nc.vector.reciprocal(s_row[:], s_ps[:].bitcast(mdt))
            rb_ps = qk_ps_pool.tile([128, NT], f32, tag="qk_ps",
                                    name=f"rbps{ti}")
            nc.tensor.matmul(rb_ps[:], lhsT=ones_row[:], rhs=s_row[:],
                             start=True, stop=True)
            rb = rb_pool.tile([128, NT], f32, tag="rb", name=f"rb{ti}")
            nc.scalar.copy(rb[:], rb_ps[:])
            for c in range(4):
                nc.vector.tensor_mul(tosbs[c][:], tosbs[c][:], rb[:])
                nc.sync.dma_start(
                    out=mem_d[c * 128:(c + 1) * 128, tnsl], in_=tosbs[c][:])

        pending_tail = None
        for i in range(NSUP):
            nsl = slice(i * NT, (i + 1) * NT)
            ro_ps = [ro_ps_pool.tile([128, NT], f32, tag=f"ro{c}",
                                     name=f"ro{c}_{i}")
                     for c in range(4)]
            sacc = sacc_pool.tile([128, NT], mdt, tag="sacc",
                                  name=f"sacc{i}")
            for t in range(MCH // 2):
                ma, mb = 2 * t, 2 * t + 1
                qk_ps = qk_ps_pool.tile([128, 2 * NT], f32, tag="qk_ps",
                                        name=f"qkps{i}_{t}")
                for h, m in ((0, ma), (1, mb)):
                    nc.tensor.matmul(
                        qk_ps[:, h * NT:(h + 1) * NT],
                        lhsT=mk_sb[:, m * 128:(m + 1) * 128],
                        rhs=qk_sb[:, nsl],
                        start=True, stop=True)
                e = e_pool.tile([128, 2 * NT], mdt, tag="E",
                                name=f"e{i}_{t}")
                nc.scalar.activation(
                    e[:], qk_ps[:], mybir.ActivationFunctionType.Exp,
                    scale=0.25)
                # sacc += g[m] * E chunk; fp32r is bit-identical to fp32,
                # the low-precision gate only keys off the dtype tag.
                with nc.allow_low_precision(reason="fp32r is fp32 bits"):
                    for h, m in ((0, ma), (1, mb)):
                        eh = e[:, h * NT:(h + 1) * NT]
                        if m == 0:
                            nc.vector.tensor_scalar_mul(
                                sacc[:], eh, g_col[:, m:m + 1])
                        else:
                            nc.vector.scalar_tensor_tensor(
                                out=sacc[:], in0=eh,
                                scalar=g_col[:, m:m + 1], in1=sacc[:],
                                op0=mybir.AluOpType.mult,
                                op1=mybir.AluOpType.add)
                if t == 2 and pending_tail is not None:
                    emit_tail(*pending_tail)
                    pending_tail = None
                for h, m in ((0, ma), (1, mb)):
                    for c in range(4):
                        nc.tensor.matmul(
                            ro_ps[c][:],
                            lhsT=mvt_sb[:, m, c * 128:(c + 1) * 128],
                            rhs=e[:, h * NT:(h + 1) * NT],
                            start=(m == 0), stop=(m == MCH - 1))

            # Evacuate readout PSUM unscaled right away so the next
            # n-super's readout matmuls get their banks back without
            # waiting on the softmax-sum/reciprocal chain.
            osbs = []
            for c in range(4):
                osb = out_pool.tile([128, NT], f32, tag="osb",
                                    name=f"osb{i}_{c}")
                nc.vector.tensor_copy(osb[:], ro_ps[c][:])
                osbs.append(osb)
            pending_tail = (i, sacc, osbs, nsl)

        emit_tail(*pending_tail)

    nc.compile()
    return nc


def _get_program():
    if "nc" not in _CACHE:
        _CACHE["nc"] = _build_program()
    return _CACHE["nc"]


def _make_in_maps(mk, qk, mv):
    mk = np.asarray(mk, dtype=np.float32)
    qk = np.asarray(qk, dtype=np.float32)
    mv = np.asarray(mv, dtype=np.float32)
    in_maps = []
    zpad = np.zeros((128 - CK, M), dtype=np.float32)
    for b in range(B):
        mk_b = np.ascontiguousarray(
            np.concatenate([mk[b].reshape(CK, M), zpad], axis=0))
        qk_b = np.ascontiguousarray(
            np.concatenate([qk[b].reshape(CK, N), zpad], axis=0))
        # mkt[p, j*CK + c] = mk[b][c, j*128 + p]
        mkt_b = np.ascontiguousarray(
            mk[b].reshape(CK, MCH, 128).transpose(2, 1, 0).reshape(
                128, MCH * CK))
        # mvt[j, p, c] = mv[b][c, j*128 + p]
        mvt_b = np.ascontiguousarray(
            mv[b].reshape(CV, MCH, 128).transpose(1, 2, 0))
        in_maps.append({"mk": mk_b, "qk": qk_b, "mkt": mkt_b, "mvt": mvt_b})
    return in_maps


def kernel(mk, qk, mv, qv):
    qv = np.asarray(qv, dtype=np.float32)
    nc = _get_program()
    from concourse.bass_utils import run_bass_kernel_spmd

    in_maps = _make_in_maps(mk, qk, mv)
    res = run_bass_kernel_spmd(nc, in_maps, list(range(N_CORES)))
    mem = np.stack([res.results[b]["mem"] for b in range(B)], axis=0)
    mem = mem.reshape(B, CV, H, W)
    return np.concatenate([mem, qv], axis=1)



# revision 2
# speedup vs baseline: 1.2964x; 1.2964x over previous
"""MemoryReader kernel for Trainium2, data-parallel over batch across 8 cores.

Per batch element b (one NeuronCore each):
    mkf = mk[b] as [CK=64, M=4096], qkf = qk[b] as [CK, N=4096]
    aff[m, n] = (2 * mkf.T @ qkf - |mkf[:,m]|^2) / sqrt(CK)
    P = softmax over m
    mem[c, n]  = sum_m mv[b][c, m] * P[m, n]
    out[b] = concat([mem, qv[b]], channel axis)

Device kernel structure (per core), v2:
    - Flat stream of 128 "pair-steps" (8 n-supers x 16 m-chunk-pairs).
      Per step: one PACKED QK slot (two concurrent K=64 matmuls via
      tile_position row-halves 0-63 / 64-127), then 8 readout matmuls.
      QK + exp are emitted ONE STEP AHEAD of the readout so the ScalarE
      exp latency is fully hidden under the readout matmul stream.
    - exp folds the -|mk|^2/8 bias per partition (bias AP from a
      host-precomputed [128, 32] table), so no on-device asq compute and
      no g-folding into mv; softmax denominator is a plain running
      tensor_add of the exp tiles.
    - Denominator tail per super: ones-matmul partition-fold (2 psum-
      accumulated MMs) -> reciprocal_approx_fast (DVE, ~5x faster than
      exact reciprocal; s is a sum of positives, no edge cases) ->
      ones-row matmul partition-broadcast -> evacuate -> 4 tensor_muls.
      Pieces are spread over steps t=1..6 of the NEXT super, with the
      two extra PSUM tiles allocated back-to-back to keep the qk-psum
      pool's 2-slot rotation parity intact (no PE stalls).
    - All matmul operands bf16 (PE rate is dtype-independent here, but
      bf16 halves DMA and enables fast weight load so the packed-QK
      LDWEIGHTS pair fits under the matmul stream); PSUM/accumulators
      stay fp32.
    - mk/qk/mv layout transforms + asq bias are host-side; qv never
      touches the device.
"""

import os
import sys

import numpy as np

B, CK, CV, H, W = 8, 64, 512, 64, 64
M = H * W          # memory positions per batch element
N = H * W          # query positions
NT = 512           # n-super-tile width (columns per softmax pass)
NSUP = N // NT     # 8 n-super-tiles
MCH = M // 128     # 32 m-chunks
PAIRS = MCH // 2   # 16 chunk-pairs per super
NSTEPS = NSUP * PAIRS
N_CORES = 8

_CACHE = {}


def _build_program():
    sys.path.insert(0, "/opt/trn_rl_repo")
    from contextlib import ExitStack

    import concourse.tile as tile
    from concourse import bacc, mybir

    dt = mybir.dt
    f32 = dt.float32
    bf16 = dt.bfloat16
    EXP = mybir.ActivationFunctionType.Exp

    nc = bacc.Bacc("TRN2", target_bir_lowering=False, debug=False,
                   num_devices=N_CORES)

    # mk2: row-packed keys. partitions 0-63 = keys of even m-chunks,
    # 64-127 = keys of odd m-chunks; free axis = (pair j, within-chunk q).
    mk2_d = nc.dram_tensor("mk2", [128, PAIRS * 128], bf16,
                           kind="ExternalInput").ap()
    # qk2: query keys duplicated into both partition halves.
    qk2_d = nc.dram_tensor("qk2", [128, N], bf16, kind="ExternalInput").ap()
    # mvt[j, p, c] = mv[c, j*128 + p]
    mvt_d = nc.dram_tensor("mvt", [MCH, 128, CV], bf16,
                           kind="ExternalInput").ap()
    # asqb[p, j] = -|mk[:, j*128+p]|^2 / 8  (exp bias per partition)
    asqb_d = nc.dram_tensor("asqb", [128, MCH], f32,
                            kind="ExternalInput").ap()
    mem_d = nc.dram_tensor("mem", [CV, N], f32, kind="ExternalOutput").ap()

    with tile.TileContext(nc) as tc, ExitStack() as ctx:
        sing = ctx.enter_context(tc.tile_pool(name="sing", bufs=1))
        e_pool = ctx.enter_context(tc.tile_pool(name="E", bufs=4))
        sacc_pool = ctx.enter_context(tc.tile_pool(name="sacc", bufs=2))
        row_pool = ctx.enter_context(tc.tile_pool(name="row", bufs=2))
        rb_pool = ctx.enter_context(tc.tile_pool(name="rb", bufs=2))
        out_pool = ctx.enter_context(tc.tile_pool(name="out", bufs=8))
        qk_ps_pool = ctx.enter_context(
            tc.tile_pool(name="qkps", bufs=2, space="PSUM"))
        ro_ps_pool = ctx.enter_context(
            tc.tile_pool(name="rops", bufs=1, space="PSUM"))

        # PE warmup: burn matmuls while input DMAs stream so the HAM
        # un-throttles (needs ~3.4us of sustained PE activity) before the
        # real matmul stream begins.
        warm_sb = sing.tile([128, 128], bf16)
        nc.vector.memset(warm_sb[:], 1.0)
        warm_ps = qk_ps_pool.tile([128, NT], f32, tag="qk_ps", name="warm_ps")
        for w in range(56):
            nc.tensor.matmul(warm_ps[:, 0:128], lhsT=warm_sb[:],
                             rhs=warm_sb[:], start=True, stop=True)

        # Resident inputs, DMA'd in dependency order: exp bias first, the
        # tensors gating the first QK matmuls, then value chunks in the
        # order the first super consumes them, then the remaining queries.
        asq_sb = sing.tile([128, MCH], f32)
        mk2_sb = sing.tile([128, PAIRS, 128], bf16)
        qk2_sb = sing.tile([128, N], bf16)
        mvt_sb = sing.tile([128, MCH, CV], bf16)
        nc.sync.dma_start(out=asq_sb[:], in_=asqb_d[:])
        nc.sync.dma_start(out=mk2_sb[:], in_=mk2_d[:].rearrange(
            "p (j q) -> p j q", q=128))
        nc.sync.dma_start(out=qk2_sb[:, 0:NT], in_=qk2_d[:, 0:NT])
        for j in range(MCH):
            nc.sync.dma_start(out=mvt_sb[:, j, :], in_=mvt_d[j])
        nc.sync.dma_start(out=qk2_sb[:, NT:N], in_=qk2_d[:, NT:N])

        ones_sb = sing.tile([128, 1], f32)
        nc.vector.memset(ones_sb[:], 1.0)
        ones_row = sing.tile([1, 128], f32)
        nc.vector.memset(ones_row[:], 1.0)

        def emit_qk(s):
            i, t = divmod(s, PAIRS)
            nsl = slice(i * NT, (i + 1) * NT)
            qp = qk_ps_pool.tile([128, 2 * NT], f32, tag="qk_ps",
                                 name=f"qkps{s}")
            # Two concurrent K=64 matmuls on row-halves (tile_position
            # auto-derives from base_partition): even chunk 2t -> cols
            # 0:NT (bank A), odd chunk 2t+1 -> cols NT:2NT (bank B).
            nc.tensor.matmul(qp[:, 0:NT], lhsT=mk2_sb[0:64, t, :],
                             rhs=qk2_sb[0:64, nsl], start=True, stop=True)
            nc.tensor.matmul(qp[:, NT:2 * NT], lhsT=mk2_sb[64:128, t, :],
                             rhs=qk2_sb[64:128, nsl], start=True, stop=True)
            return qp

        def emit_exp(s, qp):
            i, t = divmod(s, PAIRS)
            e = e_pool.tile([128, 2 * NT], bf16, tag="E", name=f"e{s}")
            for h in (0, 1):
                m = 2 * t + h
                nc.scalar.activation(
                    e[:, h * NT:(h + 1) * NT], qp[:, h * NT:(h + 1) * NT],
                    EXP, bias=asq_sb[:, m:m + 1], scale=0.25)
            return e

        qp_next = emit_qk(0)
        e_tiles = {0: emit_exp(0, qp_next)}
        prev = None          # tail state for the previous super
        ro_ps = None
        sacc2 = None

        for s in range(NSTEPS):
            i, t = divmod(s, PAIRS)
            nsl = slice(i * NT, (i + 1) * NT)
            if t == 0:
                ro_ps = [ro_ps_pool.tile([128, NT], f32, tag=f"ro{c}",
                                         name=f"ro{c}_{i}")
                         for c in range(4)]
                sacc2 = sacc_pool.tile([128, 2 * NT], f32, tag="sacc",
                                       name=f"sacc{i}")

            # QK + exp for the NEXT step (one step of software pipeline).
            if s + 1 < NSTEPS:
                qp_next = emit_qk(s + 1)

            # Tail PE pieces for the previous super. s_ps/rb_ps are
            # allocated back-to-back so the qk_ps 2-slot rotation parity
            # is preserved for subsequent QK allocations.
            if prev is not None:
                if t == 1:
                    prev["s_ps"] = qk_ps_pool.tile(
                        [1, NT], f32, tag="qk_ps", name=f"sps{i - 1}")
                    prev["rb_ps"] = qk_ps_pool.tile(
                        [128, NT], f32, tag="qk_ps", name=f"rbps{i - 1}")
                    nc.tensor.matmul(prev["s_ps"][:], lhsT=ones_sb[:],
                                     rhs=prev["sacc2"][:, 0:NT],
                                     start=True, stop=False)
                    nc.tensor.matmul(prev["s_ps"][:], lhsT=ones_sb[:],
                                     rhs=prev["sacc2"][:, NT:2 * NT],
                                     start=False, stop=True)
                elif t == 2:
                    nc.tensor.matmul(prev["rb_ps"][:], lhsT=ones_row[:],
                                     rhs=prev["s_row"][:],
                                     start=True, stop=True)

            if s + 1 < NSTEPS:
                e_tiles[s + 1] = emit_exp(s + 1, qp_next)

            # Softmax-denominator accumulation (DVE), full 1024 width.
            e = e_tiles.pop(s)
            if t == 0:
                nc.vector.tensor_copy(sacc2[:], e[:])
            else:
                nc.vector.tensor_add(sacc2[:], sacc2[:], e[:])

            # Tail DVE pieces for the previous super.
            if prev is not None:
                if t == 1:
                    prev["s_row"] = row_pool.tile([1, NT], f32, tag="srow",
                                                  name=f"srow{i - 1}")
                    nc.vector.reciprocal_approx_fast(prev["s_row"][:],
                                                     prev["s_ps"][:])
                elif t == 2:
                    prev["rb"] = rb_pool.tile([128, NT], f32, tag="rb",
                                              name=f"rb{i - 1}")
                    nc.vector.tensor_copy(prev["rb"][:], prev["rb_ps"][:])
                elif 3 <= t <= 6:
                    c = t - 3
                    osb = prev["osbs"][c]
                    nc.vector.tensor_mul(osb[:], osb[:], prev["rb"][:])
                    nc.sync.dma_start(
                        out=mem_d[c * 128:(c + 1) * 128, prev["nsl"]],
                        in_=osb[:])
                    if t == 6:
                        prev = None

            # Readout matmuls for this step.
            for h in (0, 1):
                m = 2 * t + h
                eh = e[:, h * NT:(h + 1) * NT]
                for c in range(4):
                    nc.tensor.matmul(
                        ro_ps[c][:],
                        lhsT=mvt_sb[:, m, c * 128:(c + 1) * 128],
                        rhs=eh, start=(m == 0), stop=(m == MCH - 1))

            if t == PAIRS - 1:
                # Evacuate readout PSUM unscaled right away so the next
                # super's readout matmuls get their banks back quickly.
                osbs = []
                for c in range(4):
                    osb = out_pool.tile([128, NT], f32, tag="osb",
                                        name=f"osb{i}_{c}")
                    nc.vector.tensor_copy(osb[:], ro_ps[c][:])
                    osbs.append(osb)
                prev = {"sacc2": sacc2, "osbs": osbs, "nsl": nsl}

        # Tail for the last super, inline.
        s_ps = qk_ps_pool.tile([1, NT], f32, tag="qk_ps", name="sps_last")
        rb_ps = qk_ps_pool.tile([128, NT], f32, tag="qk_ps", name="rbps_last")
        nc.tensor.matmul(s_ps[:], lhsT=ones_sb[:],
                         rhs=prev["sacc2"][:, 0:NT], start=True, stop=False)
        nc.tensor.matmul(s_ps[:], lhsT=ones_sb[:],
                         rhs=prev["sacc2"][:, NT:2 * NT],
                         start=False, stop=True)
        s_row = row_pool.tile([1, NT], f32, tag="srow", name="srow_last")
        nc.vector.reciprocal_approx_fast(s_row[:], s_ps[:])
        nc.tensor.matmul(rb_ps[:], lhsT=ones_row[:], rhs=s_row[:],
                         start=True, stop=True)
        rb = rb_pool.tile([128, NT], f32, tag="rb", name="rb_last")
        nc.vector.tensor_copy(rb[:], rb_ps[:])
        for c in range(4):
            osb = prev["osbs"][c]
            nc.vector.tensor_mul(osb[:], osb[:], rb[:])
            nc.sync.dma_start(out=mem_d[c * 128:(c + 1) * 128, prev["nsl"]],
                              in_=osb[:])

    nc.compile()
    return nc


def _get_program():
    if "nc" not in _CACHE:
        _CACHE["nc"] = _build_program()
    return _CACHE["nc"]


def _make_in_maps(mk, qk, mv):
    import ml_dtypes

    bf16 = ml_dtypes.bfloat16
    mk = np.asarray(mk, dtype=np.float32)
    qk = np.asarray(qk, dtype=np.float32)
    mv = np.asarray(mv, dtype=np.float32)
    in_maps = []
    for b in range(B):
        mkf = mk[b].reshape(CK, M)
        # mk2: [64 even-chunk keys; 64 odd-chunk keys] x (pair, q)
        mk3 = mkf.reshape(CK, PAIRS, 2, 128)
        mk2 = np.concatenate([mk3[:, :, 0, :], mk3[:, :, 1, :]],
                             axis=0).reshape(128, PAIRS * 128)
        qkf = qk[b].reshape(CK, N)
        qk2 = np.concatenate([qkf, qkf], axis=0)
        mvt = np.ascontiguousarray(
            mv[b].reshape(CV, MCH, 128).transpose(1, 2, 0))
        asq = (mkf * mkf).sum(axis=0)                     # [M]
        asqb = np.ascontiguousarray(
            asq.reshape(MCH, 128).T * np.float32(-0.125))
        in_maps.append({
            "mk2": np.ascontiguousarray(mk2).astype(bf16),
            "qk2": np.ascontiguousarray(qk2).astype(bf16),
            "mvt": mvt.astype(bf16),
            "asqb": asqb.astype(np.float32),
        })
    return in_maps


def kernel(mk, qk, mv, qv):
    qv = np.asarray(qv, dtype=np.float32)
    nc = _get_program()
    from concourse.bass_utils import run_bass_kernel_spmd

    in_maps = _make_in_maps(mk, qk, mv)
    res = run_bass_kernel_spmd(nc, in_maps, list(range(N_CORES)))
    mem = np.stack([res.results[b]["mem"] for b in range(B)], axis=0)
    mem = mem.reshape(B, CV, H, W)
    return np.concatenate([mem, qv], axis=1)
